# revision 40
# baseline (speedup 1.0000x reference)
"""Bass/Trainium2 kernel for nn_BatchLoreAttentionLayer (fp8 DoubleRow version).

Reference math (per batch item b, X = embeddings[b] in [L=128, D=256]):
    Q = X q_w^T + q_b ; K = X k_w^T + k_b
    S = Q K^T / sqrt(D) ; S[:, padded] = -inf
    attn = softmax_m(S) ; attended = attn X
    out = tanh( (valid^T attended) / cnt )

Algebraic restructure (q_b == 0 always holds for this module's inputs;
k_b drops inside the softmax):
    S = X A X^T,  A = q_w^T k_w / sqrt(D)
    Padded ROWS and COLUMNS of X are zeroed host-side, so S[l, m_pad] = 0
    exactly and E = exp(S) is exactly 1 in padded columns; the softmax
    denominator is fixed up by subtracting npad (per-item padded count).
    w[m] = sum_l g[l] E[l,m],  g = (2^9 valid/cnt) / (rowsum(E) - npad)
    out = tanh( 2^-9 * X^T w )    (padded rows of X are zero, so no mask
                                   is needed on w)

Device pipeline per 4-item group (explicit 20-stage software pipeline;
every cross-engine edge is >= 1 iteration old so no in-order SEQ blocks):
    Yt = A'^T Xt       2 fp8e4 DoubleRow matmuls (A' = 2^6 A, 0.5 cyc/col
                       at 256-deep contraction) into one 2-bank PSUM tile
    Yt PSUM->SBUF fp8  split ACT (cols 0:192) / DVE (192:512) x2 e-blocks
                       (GPSIMD must not touch PSUM on real HW)
    S  = Yt^T Xt       4 fp8 DoubleRow matmuls (per item)
    E  = exp(2^-6 S)   1 ACT op -> bf16
    rowsum tree        halving add on Pool (SBUF), second halving on Pool,
                       final reduce_sum + (rs - npad) pair + 1/rs on DVE/Pool
    g  = rinv * vt'    Pool -> bf16 (pair-batched)
    w  = E^T g         4 N=1 bf16 matmuls into chunk-wide PSUM columns
    oT += xl^T w       8 N=1 bf16 matmuls into the same chunk PSUM bank
    tanh(2^-9 oT)      once per 128-item chunk, then output DMA
DMA: xt fp8 runs one 8-group slab ahead of xl bf16 (xl is only needed by
the final oT matmuls); big slab DMAs amortize the ~625ns HWDGE cost.
Cost model: ~71.5us DMA / ~70us ACT+DVE+Pool per core -> 88.6us makespan.
Accuracy: xl/w/g/E in bf16, xt/Yt/A in fp8e4 -> rel err ~1.4e-2 (< 2e-2);
fp8 anywhere in the output path fails the tolerance, which is why xl
stays bf16 (this sets the DMA floor).

Sharding: pure data-parallel over B across 8 cores (256 items/core).
"""

import sys
from contextlib import ExitStack

import numpy as np
import ml_dtypes

sys.path.insert(0, "/opt/trn_rl_repo")

import concourse.bass as bass  # noqa: E402
import concourse.mybir as mybir  # noqa: E402
import concourse.tile as tile  # noqa: E402
from concourse import bacc  # noqa: E402
from concourse.bass import ts  # noqa: E402
from concourse.bass_utils import run_bass_kernel_spmd  # noqa: E402

B, L, D = 2048, 128, 256
NCORES = 8
BPC = B // NCORES   # items per core
GRP = 4             # items per group
NG = BPC // GRP     # groups per core (64)
SLAB = 8            # groups per DMA slab (amortizes ~625ns HWDGE per DMA)
NS = NG // SLAB     # slabs per core (8)
CHUNK = 128         # items per output chunk
GPC = CHUNK // GRP  # groups per chunk (32)

SA = 6   # A scale exponent (A' = 2^SA * A)
SG = 9   # g scale exponent (vt' = 2^SG * valid/cnt)

F32 = mybir.dt.float32
BF16 = mybir.dt.bfloat16
F8 = mybir.dt.float8e4
AF = mybir.ActivationFunctionType
DR = mybir.MatmulPerfMode.DoubleRow

NPF8 = ml_dtypes.float8_e4m3
NPBF = ml_dtypes.bfloat16

_CACHE = {}
_STAGE = [None]
import os as _os
D0 = int(_os.environ.get("K_D0", "192"))     # ACT cols (per e-block) of yt copy
LAG_S = int(_os.environ.get("K_LS", "7"))
LAG_E = LAG_S + 1          # exp (ACT)
LAG_T1 = int(_os.environ.get("K_LT1", "9"))   # tt-add half (Pool)
LAG_T2 = LAG_T1 + 1        # tt-add quarter (Pool)
LAG_R = LAG_T1 + 2         # final reduce (DVE)
LAG_SB = int(_os.environ.get("K_LSB", "13"))  # sub pair (Pool)
LAG_RC = LAG_SB + 1        # recip pair (DVE)
LAG_G = int(_os.environ.get("K_LG", "15"))    # gmul pair (Pool)
LAG_W = int(_os.environ.get("K_LW", "16"))
LAG_WC = int(_os.environ.get("K_LWC", "18"))  # wcopy pair (DVE)
LAG_O = int(_os.environ.get("K_LO", "19"))  # debug: (stage_name, group) during build


def build_bass():
    nc = bacc.Bacc(None, target_bir_lowering=False)
    xt = nc.declare_dram_parameter(
        "xt", [NS, 128, SLAB * 2 * GRP * L], F8, isOutput=False
    )
    xl = nc.declare_dram_parameter(
        "xl", [NS, 128, SLAB * GRP * D], BF16, isOutput=False
    )
    aw = nc.declare_dram_parameter("aw", [128, 2, D], F8, isOutput=False)
    vt = nc.declare_dram_parameter("vt", [L, BPC], F32, isOutput=False)
    npd = nc.declare_dram_parameter("npd", [128, BPC], BF16, isOutput=False)
    outT = nc.declare_dram_parameter("outT", [2, 128, BPC], F32, isOutput=True)
    build_body(nc, xt, xl, aw, vt, npd, outT)
    nc.finalize()
    return nc


def build_body(nc, xt, xl, aw, vt, npd, outT):
    with tile.TileContext(nc) as tc, ExitStack() as ctx:
        singles = ctx.enter_context(tc.tile_pool(name="singles", bufs=1))
        io = ctx.enter_context(tc.tile_pool(name="io", bufs=7))
        work = ctx.enter_context(tc.tile_pool(name="work", bufs=3))
        small = ctx.enter_context(tc.tile_pool(name="small", bufs=4))
        ps_yt = ctx.enter_context(tc.tile_pool(name="ps_yt", bufs=2, space="PSUM"))
        ps_s = ctx.enter_context(tc.tile_pool(name="ps_s", bufs=2, space="PSUM"))
        ps_wo = ctx.enter_context(tc.tile_pool(name="ps_wo", bufs=2, space="PSUM"))

        # ---- PE clock warmup: ~3us of junk matmuls during the first DMA ----
        warm = singles.tile([128, 128], BF16)
        nc.vector.memset(warm, 0.0)
        nc.scalar.activation(out=warm, in_=warm, func=AF.Exp)
        warm_ps = ps_yt.tile([128, 512], F32, tag="yt", name="warm_ps")
        for _ in range(15):
            nc.tensor.matmul(
                out=warm_ps[:, 0:128], lhsT=warm, rhs=warm, start=True, stop=True
            )

        # per-slab / per-group tile handles
        xt_slabs = [None] * NS
        xl_slabs = [None] * NS
        yts = [None] * NG
        s4s = [None] * NG
        e4s = [None] * NG
        rs4s = [None] * NG
        rsc4s = [None] * NG
        rinv4s = [None] * NG
        g4s = [None] * NG
        w4s = [None] * NG
        wo_ps = [None, None]     # per chunk: [:,0,:]=wcol, [:,1:3,:]=oT

        def dma_xt_slab(s):
            t = io.tile([128, SLAB * 2 * GRP * L], F8, tag="xt", bufs=4,
                        name=f"xts_{s}")
            if s == 0:
                # split slab 0 so the pipeline starts after a 2-group DMA
                q = 2 * 2 * GRP * L
                nc.sync.dma_start(out=t[:, 0:q], in_=xt[0][:, 0:q])
                nc.sync.dma_start(out=t[:, q:], in_=xt[0][:, q:])
            else:
                nc.sync.dma_start(out=t, in_=xt[s])
            # [d_lo(p), group-in-slab, d_hi(t), (j l)]
            xt_slabs[s] = t.rearrange("p (k t jl) -> p k t jl", k=SLAB, t=2)

        def dma_xl_slab(s):
            t2 = io.tile([128, SLAB * GRP * D], BF16, tag="xl", bufs=5,
                         name=f"xls_{s}")
            # halves: consumers of the first 4 groups unblock ~2.9us earlier
            # (matters most for the final slab's drain)
            h = SLAB * GRP * D // 2
            nc.sync.dma_start(out=t2[:, 0:h], in_=xl[s][:, 0:h])
            nc.sync.dma_start(out=t2[:, h:], in_=xl[s][:, h:])
            xl_slabs[s] = t2.rearrange("p (k j d) -> p k j d", k=SLAB, j=GRP)

        def xt4(g):
            return xt_slabs[g // SLAB][:, g % SLAB]   # [d_lo, d_hi, (j l)]

        def xl4(g):
            return xl_slabs[g // SLAB][:, g % SLAB]   # [m, item, d]

        yt_pss = [None] * NG

        def stage_yt_mm(g):
            # one 2-bank PSUM tile for both e-blocks (single-AP copies)
            yt_ps = ps_yt.tile([128, 2, GRP * L], F32, tag="yt", name=f"ytp_{g}")
            for eb in range(2):
                nc.tensor.matmul(
                    out=yt_ps[:, eb, :],
                    lhsT=a_sb[:, :, ts(eb, 128)],
                    rhs=xt4(g),
                    start=True,
                    stop=True,
                    perf_mode=DR,
                )
            yt_pss[g] = yt_ps

        def stage_yt_copy(g):
            # GPSIMD cannot access PSUM on real HW: split ACT / DVE only
            yt_ps = yt_pss[g]
            yt_sb = work.tile([128, 2, GRP * L], F8, tag="yt", bufs=5, name=f"yt_{g}")
            nc.scalar.activation(
                out=yt_sb[:, :, 0:D0], in_=yt_ps[:, :, 0:D0], func=AF.Copy
            )
            nc.vector.tensor_copy(out=yt_sb[:, :, D0:512], in_=yt_ps[:, :, D0:512])
            yts[g] = yt_sb.rearrange("p t (j l) -> p t j l", j=GRP)

        def stage_s_mm(g):
            s4 = ps_s.tile([128, GRP, L], F32, tag="s4", name=f"s4_{g}")
            for j in range(GRP):
                nc.tensor.matmul(
                    out=s4[:, j, :],
                    lhsT=yts[g][:, :, j, :],
                    rhs=xt4(g)[:, :, ts(j, L)],
                    start=True,
                    stop=True,
                    perf_mode=DR,
                )
            s4s[g] = s4

        def stage_exp(g):
            e4 = work.tile([128, GRP, L], BF16, tag="e4", bufs=10, name=f"e4_{g}")
            nc.scalar.activation(
                out=e4, in_=s4s[g], func=AF.Exp, scale=float(2.0**-SA)
            )
            e4s[g] = e4

        t1s = [None] * NG
        t2s = [None] * NG

        def stage_tt1(g):    # Pool: first halving of the row-sum tree (SBUF)
            t1 = small.tile([128, GRP, 64], BF16, tag="t1", name=f"t1_{g}")
            nc.gpsimd.tensor_add(t1, e4s[g][:, :, 0:64], e4s[g][:, :, 64:128])
            t1s[g] = t1

        def stage_tt2(g):    # Pool: second halving (SBUF only)
            t2 = small.tile([128, GRP, 32], BF16, tag="t2", name=f"t2_{g}")
            nc.gpsimd.tensor_add(t2, t1s[g][:, :, 0:32], t1s[g][:, :, 32:64])
            t2s[g] = t2

        def stage_reduce(g):
            # rs for a PAIR of groups lives in one [128, 8] tile (halves
            # written by consecutive reduce calls) so the downstream small
            # ops batch two groups per instruction.
            if g % 2 == 0:
                rs4s[g // 2] = small.tile(
                    [128, 2 * GRP], F32, tag="rs8", name=f"rs8_{g}"
                )
            half = rs4s[g // 2][:, (g % 2) * GRP : (g % 2 + 1) * GRP]
            nc.vector.reduce_sum(out=half, in_=t2s[g], axis=mybir.AxisListType.X)

        def stage_sub(g):    # pair op, g even (Pool, SBUF only)
            i0 = g * GRP
            rsc = small.tile([128, 2 * GRP], F32, tag="rsc8", name=f"rsc8_{g}")
            nc.gpsimd.tensor_sub(rsc, rs4s[g // 2], npd_sb[:, i0 : i0 + 2 * GRP])
            rsc4s[g // 2] = rsc

        def stage_recip(g):  # pair op, g even
            rinv = small.tile([128, 2 * GRP], F32, tag="rinv8", name=f"rinv8_{g}")
            nc.vector.reciprocal(out=rinv, in_=rsc4s[g // 2])
            rinv4s[g // 2] = rinv

        def stage_gmul(g):   # pair op, g even
            i0 = g * GRP
            g8 = small.tile([128, 2 * GRP], BF16, tag="g8", name=f"g8_{g}")
            nc.gpsimd.tensor_mul(g8, rinv4s[g // 2], vt_sb[:, i0 : i0 + 2 * GRP])
            g4s[g // 2] = g8

        def stage_w_mm(g):
            c = g // GPC
            if g % GPC == 0:
                wo_ps[c] = ps_wo.tile([128, 3, CHUNK], F32, tag="wo",
                                      name=f"wo_{c}")
            col0 = (g * GRP) % CHUNK
            goff = (g % 2) * GRP
            for j in range(GRP):
                nc.tensor.matmul(
                    out=wo_ps[c][:, 0, col0 + j : col0 + j + 1],
                    lhsT=e4s[g][:, j, :],
                    rhs=g4s[g // 2][:, goff + j : goff + j + 1],
                    start=True,
                    stop=True,
                )

        def stage_wcopy(g):  # pair op, g even; padded xl rows are zero, so
            c = g // GPC     # no valid-mask multiply is needed on w
            col0 = (g * GRP) % CHUNK
            w8 = small.tile([128, 2 * GRP], BF16, tag="w8", name=f"w8_{g}")
            nc.vector.tensor_copy(out=w8, in_=wo_ps[c][:, 0, col0 : col0 + 2 * GRP])
            w4s[g // 2] = w8

        def stage_oT_mm(g):
            c = g // GPC
            col0 = (g * GRP) % CHUNK
            goff = (g % 2) * GRP
            for j in range(GRP):
                for dh in range(2):
                    nc.tensor.matmul(
                        out=wo_ps[c][:, 1 + dh, col0 + j : col0 + j + 1],
                        lhsT=xl4(g)[:, j, ts(dh, 128)],
                        rhs=w4s[g // 2][:, goff + j : goff + j + 1],
                        start=True,
                        stop=True,
                    )
            if g % GPC == GPC - 1:
                oT_sb = work.tile([128, 2, CHUNK], F32, tag="oT_sb", bufs=2,
                                  name=f"oTsb_{c}")
                nc.scalar.activation(
                    out=oT_sb, in_=wo_ps[c][:, 1:3, :], func=AF.Tanh,
                    scale=float(2.0**-SG)
                )
                for dh in range(2):
                    nc.sync.dma_start(
                        out=outT[dh, :, c * CHUNK : (c + 1) * CHUNK],
                        in_=oT_sb[:, dh, :],
                    )

        # Deep software pipeline: every cross-engine dependency is >= 1
        # iteration old so no engine SEQ ever waits on same-iteration work.
        # (lag, fn, pair) -- pair ops fire when (i - lag) is even.
        STAGES = (
            (4, stage_yt_mm, False),    # PE
            (5, stage_yt_copy, False),  # ACT+DVE split, first in each stream
            (LAG_WC, stage_wcopy, True),  # DVE (PSUM read)
            (LAG_S, stage_s_mm, False),   # PE
            (LAG_O, stage_oT_mm, False),  # PE: old-chunk oT before new w
            (LAG_W, stage_w_mm, False),   # PE
            (LAG_E, stage_exp, False),    # ACT
            (LAG_T1, stage_tt1, False),   # Pool (SBUF halving add)
            (LAG_T2, stage_tt2, False),   # DVE
            (LAG_R, stage_reduce, False), # DVE
            (LAG_SB, stage_sub, True),    # Pool
            (LAG_RC, stage_recip, True),  # DVE
            (LAG_G, stage_gmul, True),    # Pool
        )
        # DMA order: xt slab 0 first (gates the whole pipeline), then the
        # small aux tensors, then xt runs one slab ahead of xl so the
        # xt-side compute chain never waits on the big bf16 xl stream.
        _STAGE[0] = ("dma", 0)
        dma_xt_slab(0)
        a_sb = singles.tile([128, 2, D], F8)      # [d_lo(p), d_hi(t), e]
        nc.sync.dma_start(out=a_sb, in_=aw[:, :, :])
        vt_sb = singles.tile([L, BPC], F32)       # 2^SG * valid/cnt, [l, b]
        nc.sync.dma_start(out=vt_sb, in_=vt[:, :])
        npd_sb = singles.tile([128, BPC], BF16)   # npad counts (exact in bf16)
        nc.sync.dma_start(out=npd_sb, in_=npd[:, :])
        for i in range(NG + LAG_O + 1):
            if i % SLAB == 0:
                s = i // SLAB
                _STAGE[0] = ("dma", s)
                if s + 1 < NS:
                    dma_xt_slab(s + 1)
                if s < NS:
                    dma_xl_slab(s)
            for lag, fn, pair in STAGES:
                g = i - lag
                if 0 <= g < NG and (not pair or g % 2 == 0):
                    _STAGE[0] = (fn.__name__, g)
                    fn(g)
        _STAGE[0] = None


def _numpy_fallback(embeddings, padding_mask, q_w, q_b, k_w, k_b):
    emb = np.asarray(embeddings, np.float32)
    mask = np.asarray(padding_mask)
    Q = emb @ np.asarray(q_w, np.float32).T + np.asarray(q_b, np.float32)
    K = emb @ np.asarray(k_w, np.float32).T + np.asarray(k_b, np.float32)
    S = np.einsum("ble,bme->blm", Q, K) / np.sqrt(np.float32(D))
    S = np.where(mask[:, None, :], -np.inf, S)
    S = S - S.max(axis=-1, keepdims=True)
    E = np.exp(S)
    attn = E / E.sum(axis=-1, keepdims=True)
    att = np.einsum("blm,bmd->bld", attn, emb)
    valid = (~mask).astype(np.float32)
    summed = np.einsum("bld,bl->bd", att, valid)
    cnt = np.maximum(valid.sum(1, keepdims=True), 1.0)
    return np.tanh(summed / cnt).astype(np.float32)


def prep_inputs(embeddings, padding_mask, q_w, q_b, k_w, k_b):
    emb = np.asarray(embeddings, np.float32)
    mask = np.asarray(padding_mask)
    q_w = np.asarray(q_w, np.float32)
    k_w = np.asarray(k_w, np.float32)
    scale = 1.0 / np.sqrt(np.float32(D))

    A = (q_w.T @ k_w) * scale * (2.0**SA)            # [D(d), D(e)]
    aw = np.ascontiguousarray(
        A.reshape(2, 128, D).transpose(1, 0, 2)       # [d_lo, d_hi, e]
    ).astype(NPF8)

    valid = (~mask).astype(np.float32)                # [B, L]
    cnt = np.maximum(valid.sum(1, keepdims=True), 1.0)
    npad = mask.sum(1).astype(np.float32)             # [B]
    vt_full = ((2.0**SG) * valid / cnt).T             # [L, B] f32
    vm_full = np.ascontiguousarray(valid.T)           # [m, B]
    npd_full = np.ascontiguousarray(
        np.broadcast_to(npad[None, :], (128, B))
    ).astype(NPBF)

    Xz = emb * valid[:, :, None]                      # zero padded rows
    X8 = Xz.astype(NPF8)
    Xb = Xz.astype(NPBF)
    # xt: [nslab, 128(d_lo), 8(k), 2(d_hi), 4(j), 128(l)] fp8
    xt8 = np.ascontiguousarray(
        X8.transpose(0, 2, 1)                         # [B, D, L]
        .reshape(B // (SLAB * GRP), SLAB, GRP, 2, 128, L)  # [s, k, j, t, p, l]
        .transpose(0, 4, 1, 3, 2, 5)                  # [s, p, k, t, j, l]
        .reshape(B // (SLAB * GRP), 128, SLAB * 2 * GRP * L)
    )
    # xl: [nslab, 128(m), 8(k), 4(j), 256(d)] bf16
    xlb = np.ascontiguousarray(
        Xb.reshape(B // (SLAB * GRP), SLAB, GRP, L, D)  # [s, k, j, m, d]
        .transpose(0, 3, 1, 2, 4)                     # [s, m, k, j, d]
        .reshape(B // (SLAB * GRP), 128, SLAB * GRP * D)
    )

    in_maps = []
    for c in range(NCORES):
        sl = slice(c * BPC, (c + 1) * BPC)
        in_maps.append(
            {
                "xt": xt8[c * NS : (c + 1) * NS],
                "xl": xlb[c * NS : (c + 1) * NS],
                "aw": aw,
                "vt": np.ascontiguousarray(vt_full[:, sl]),
                "npd": np.ascontiguousarray(npd_full[:, sl]),
                "vm": np.ascontiguousarray(vm_full[:, sl]),
            }
        )
    return in_maps


def _get_nc():
    if "nc" not in _CACHE:
        _CACHE["nc"] = build_bass()
    return _CACHE["nc"]


def _make_exec():
    """Build the shard_map'd PJRT executable once (mirrors
    bass2jax.run_bass_via_pjrt, but returns a reusable callable)."""
    import jax
    from jax.sharding import Mesh, PartitionSpec
    from jax.experimental.shard_map import shard_map
    from concourse import bass2jax, mybir as _mybir

    nc = _get_nc()
    bass2jax.install_neuronx_cc_hook()
    partition_name = nc.partition_id_tensor.name if nc.partition_id_tensor else None
    in_names, out_names, out_avals, zero_outs = [], [], [], []
    for alloc in nc.m.functions[0].allocations:
        if not isinstance(alloc, _mybir.MemoryLocationSet):
            continue
        name = alloc.memorylocations[0].name
        if alloc.kind == "ExternalInput":
            if name != partition_name:
                in_names.append(name)
        elif alloc.kind == "ExternalOutput":
            shape = tuple(alloc.tensor_shape)
            dtype = _mybir.dt.np(alloc.dtype)
            out_names.append(name)
            out_avals.append(jax.core.ShapedArray(shape, dtype))
            zero_outs.append(np.zeros(shape, dtype))
    n_params = len(in_names)
    in_names_full = in_names + out_names
    if partition_name is not None:
        in_names_full.append(partition_name)

    def _body(*args):
        operands = list(args)
        if partition_name is not None:
            operands.append(bass2jax.partition_id_tensor())
        outs = bass2jax._bass_exec_p.bind(
            *operands,
            out_avals=tuple(out_avals),
            in_names=tuple(in_names_full),
            out_names=tuple(out_names),
            lowering_input_output_aliases=(),
            sim_require_finite=True,
            sim_require_nnan=True,
            nc=nc,
        )
        return tuple(outs)

    devices = jax.devices()[:NCORES]
    mesh = Mesh(np.asarray(devices), ("core",))
    n_outs = len(out_names)
    sharded = jax.jit(
        shard_map(
            _body,
            mesh=mesh,
            in_specs=(PartitionSpec("core"),) * (n_params + n_outs),
            out_specs=(PartitionSpec("core"),) * n_outs,
            check_rep=False,
        ),
        donate_argnums=tuple(range(n_params, n_params + n_outs)),
        keep_unused=True,
    )

    def run(in_maps, n_iters=1, timings=None):
        import time as _t

        concat_in = [
            np.concatenate([np.asarray(in_maps[c][nm]) for c in range(NCORES)], axis=0)
            for nm in in_names
        ]
        placed = [jax.device_put(a) for a in concat_in]
        zo = [np.concatenate([z] * NCORES, axis=0) for z in zero_outs]
        outs = None
        for _ in range(n_iters):
            zplaced = [jax.device_put(z) for z in zo]
            for p in placed + zplaced:
                p.block_until_ready()
            t0 = _t.perf_counter()
            outs = sharded(*placed, *zplaced)
            for o in outs:
                o.block_until_ready()
            if timings is not None:
                timings.append(_t.perf_counter() - t0)
        res = []
        for c in range(NCORES):
            d = {}
            for i, nm in enumerate(out_names):
                full = np.asarray(outs[i])
                per = full.shape[0] // NCORES
                d[nm] = full[c * per : (c + 1) * per]
            res.append(d)
        return res

    return run


def _get_runner():
    if "run" not in _CACHE:
        _CACHE["run"] = _make_exec()
    return _CACHE["run"]


def kernel(embeddings, padding_mask, q_w, q_b, k_w, k_b, _n_iters=None, _timings=None):
    # q_b folds into an extra per-key bias this kernel doesn't model;
    # setup_inputs always passes q_b == 0 (k_b cancels inside softmax).
    if np.any(np.asarray(q_b)):
        return _numpy_fallback(embeddings, padding_mask, q_w, q_b, k_w, k_b)
    in_maps = prep_inputs(embeddings, padding_mask, q_w, q_b, k_w, k_b)
    if _n_iters is None:
        res = run_bass_kernel_spmd(_get_nc(), in_maps, list(range(NCORES)))
        results = res.results
    else:
        results = _get_runner()(in_maps, n_iters=_n_iters, timings=_timings)
    out = np.empty((B, D), np.float32)
    for c in range(NCORES):
        oT = np.asarray(results[c]["outT"], np.float32)  # [2, 128, BPC]
        out[c * BPC : (c + 1) * BPC] = oT.reshape(D, BPC).T
    return out


if __name__ == "__main__":
    ref_inputs = {
        "embeddings": np.random.randn(B, L, D).astype(np.float32),
        "padding_mask": np.random.rand(B, L) < 0.3,
        "q_w": np.random.randn(D, D).astype(np.float32) * 0.06,
        "q_b": np.zeros(D, np.float32),
        "k_w": np.random.randn(D, D).astype(np.float32) * 0.06,
        "k_b": np.zeros(D, np.float32),
    }
    out = kernel(**ref_inputs)
    print(out.shape, out.dtype)
